# revision 1
# baseline (speedup 1.0000x reference)
"""nn_AttNet kernel for 8 TRN2 NeuronCores (SPMD via bass).

Pipeline (BS=2, T=3, C=7, N=130000; BEV 512x512, RV 64x2048, Fp=64):
  A (device, 8 cores, point-sharded): h = relu(w2 @ relu(w1 @ pf))
  host: scatter-max into BEV/RV grids (sort + maximum.reduceat), residual stack
  B (device, 8 cores, (sample, band)-sharded): 3x3 convs as matmul streams
  host: grid->point gathers
  C (device, 8 cores, point-sharded): fusion MLP -> pred
"""
import math
from contextlib import ExitStack

import numpy as np
import ml_dtypes

import concourse.bass as bass
import concourse.tile as tile
from concourse import bacc, mybir
from concourse.bass_utils import run_bass_kernel_spmd

BS, T, C, N = 2, 3, 7, 130000
FP = 64
BEV_H, BEV_W = 512, 512
RV_H, RV_W = 64, 2048
NCORES = 8

F32 = mybir.dt.float32
F32R = mybir.dt.float32r
BF16 = mybir.dt.bfloat16

_total_exec_ns = [0.0]
_launch_wall_ns = [0.0]


import os
_TRACE = os.environ.get("KERNEL_TRACE", "0") == "1"


def _run(nc, in_maps):
    import time as _time
    nc.compile()
    t0 = _time.time()
    res = run_bass_kernel_spmd(nc, in_maps, list(range(len(in_maps))),
                               trace=_TRACE)
    _launch_wall_ns[0] += (_time.time() - t0) * 1e9
    if res.exec_time_ns:
        _total_exec_ns[0] += res.exec_time_ns
    return res.results


# ---------------------------------------------------------------- launch A
def _build_mlp(np_pts, cols):
    """h[64, np_pts] = relu(w2 @ relu(w1 @ pf)); f32 end to end."""
    nc = bacc.Bacc("TRN2", target_bir_lowering=False)
    pf = nc.dram_tensor("pf", [C, np_pts], BF16, kind="ExternalInput")
    w1t = nc.dram_tensor("w1t", [C, FP], BF16, kind="ExternalInput")
    w2t = nc.dram_tensor("w2t", [FP, FP], BF16, kind="ExternalInput")
    h_out = nc.dram_tensor("h", [FP, np_pts], F32, kind="ExternalOutput")
    CH = 8192
    nchunk = np_pts // CH
    with tile.TileContext(nc) as tc:
        with ExitStack() as ctx:
            sb = ctx.enter_context(tc.tile_pool(name="sb", bufs=2))
            wp = ctx.enter_context(tc.tile_pool(name="wp", bufs=1))
            ps = ctx.enter_context(tc.tile_pool(name="ps", bufs=4, space="PSUM"))
            w1s = wp.tile([C, FP], BF16)
            nc.sync.dma_start(out=w1s[:], in_=w1t[:])
            w2s = wp.tile([FP, FP], BF16)
            nc.sync.dma_start(out=w2s[:], in_=w2t[:])
            for ci in range(nchunk):
                x = sb.tile([C, CH], BF16, tag="x")
                nc.sync.dma_start(out=x[:], in_=pf[:, bass.ts(ci, CH)])
                hchunk = sb.tile([FP, CH], F32, tag="h")
                for t in range(CH // 512):
                    p1 = ps.tile([FP, 512], F32, tag="p1")
                    nc.tensor.matmul(out=p1[:], lhsT=w1s[:],
                                     rhs=x[:, bass.ts(t, 512)],
                                     start=True, stop=True)
                    h1 = sb.tile([FP, 512], BF16, tag="h1")
                    nc.scalar.activation(h1[:], p1[:],
                                         mybir.ActivationFunctionType.Relu)
                    p2 = ps.tile([FP, 512], F32, tag="p2")
                    nc.tensor.matmul(out=p2[:], lhsT=w2s[:],
                                     rhs=h1[:],
                                     start=True, stop=True)
                    nc.scalar.activation(hchunk[:, bass.ts(t, 512)], p2[:],
                                         mybir.ActivationFunctionType.Relu)
                nc.sync.dma_start(out=h_out[:, bass.ts(ci, CH)], in_=hchunk[:])
    return nc


# ---------------------------------------------------------------- launch B
def _build_conv(rows_out, halo_rows, width, cin_lo, cin_hi, cout, nrv_rows,
                rv_halo, rv_w):
    """BEV band conv (cin=192 split 128+64) + RV band conv (cin=64).

    inputs (channel-major, f32->bf16 on host):
      glo [128, halo_rows, width], ghi [64, halo_rows, width]  (BEV band)
      klo [128, 9, 64], khi [64, 9, 64]                        (BEV taps)
      grv [64, rv_halo, rv_w], krv [64, 9, 64]                 (RV)
    outputs: bf [64, rows_out, width] f32, rf [64, nrv_rows, rv_w] f32
    """
    nc = bacc.Bacc("TRN2", target_bir_lowering=False)
    glo = nc.dram_tensor("glo", [cin_lo, halo_rows, width], BF16, kind="ExternalInput")
    ghi = nc.dram_tensor("ghi", [2 * cin_hi, halo_rows // 2, width], BF16, kind="ExternalInput")
    klo = nc.dram_tensor("klo", [cin_lo, 4, 3, 2 * cout], BF16, kind="ExternalInput")
    khi = nc.dram_tensor("khi", [2 * cin_hi, 2, 3, 2 * cout], BF16, kind="ExternalInput")
    grv = nc.dram_tensor("grv", [128, rv_halo // 2, rv_w], BF16, kind="ExternalInput")
    krv = nc.dram_tensor("krv", [128, 2, 3, 2 * cout], BF16, kind="ExternalInput")
    bf = nc.dram_tensor("bf", [2 * cout, rows_out // 2, width], F32, kind="ExternalOutput")
    rf = nc.dram_tensor("rf", [2 * cout, nrv_rows // 2, rv_w], F32, kind="ExternalOutput")

    RCH = 8             # output rows per chunk
    with tile.TileContext(nc) as tc:
        with ExitStack() as ctx:
            wp = ctx.enter_context(tc.tile_pool(name="wp", bufs=1))
            sb = ctx.enter_context(tc.tile_pool(name="sb", bufs=2))
            rvp = ctx.enter_context(tc.tile_pool(name="rvp", bufs=1))
            ob = ctx.enter_context(tc.tile_pool(name="ob", bufs=2))
            ps = ctx.enter_context(tc.tile_pool(name="ps", bufs=4, space="PSUM"))
            klos = wp.tile([cin_lo, 4, 3, 2 * cout], BF16)
            nc.sync.dma_start(out=klos[:], in_=klo[:])
            khis = wp.tile([2 * cin_hi, 2, 3, 2 * cout], BF16)
            nc.sync.dma_start(out=khis[:], in_=khi[:])
            krvs = wp.tile([128, 2, 3, 2 * cout], BF16)
            nc.sync.dma_start(out=krvs[:], in_=krv[:])

            # BEV band: chunks of RCH output rows, 2 rows packed in M
            for ch in range(rows_out // RCH):
                in0 = ch * RCH
                nrows_in = RCH + 2
                tlo = sb.tile([cin_lo, nrows_in, width], BF16, tag="tlo")
                nc.sync.dma_start(out=tlo[:], in_=glo[:, in0:in0 + nrows_in, :])
                thi = sb.tile([2 * cin_hi, nrows_in // 2, width], BF16, tag="thi")
                nc.sync.dma_start(out=thi[:],
                                  in_=ghi[:, in0 // 2:(in0 + nrows_in) // 2, :])
                outc = ob.tile([2 * cout, RCH // 2, width], F32, tag="outc")
                for pr in range(RCH // 2):
                    r = 2 * pr
                    acc = ps.tile([2 * cout, width], F32, tag="acc")
                    nmm = 0
                    for j in range(4):
                        for dx in range(3):
                            if dx == 0:
                                dst, src = slice(1, width), slice(0, width - 1)
                            elif dx == 2:
                                dst, src = slice(0, width - 1), slice(1, width)
                            else:
                                dst, src = slice(0, width), slice(0, width)
                            nc.tensor.matmul(
                                out=acc[:, dst], lhsT=klos[:, j, dx, :],
                                rhs=tlo[:, r + j, src],
                                start=(nmm == 0), stop=False)
                            nmm += 1
                    for pa in range(2):
                        for dx in range(3):
                            if dx == 0:
                                dst, src = slice(1, width), slice(0, width - 1)
                            elif dx == 2:
                                dst, src = slice(0, width - 1), slice(1, width)
                            else:
                                dst, src = slice(0, width), slice(0, width)
                            nc.tensor.matmul(
                                out=acc[:, dst], lhsT=khis[:, pa, dx, :],
                                rhs=thi[:, pr + pa, src],
                                start=False, stop=(nmm == 17))
                            nmm += 1
                    nc.scalar.activation(outc[:, pr, :], acc[:],
                                         mybir.ActivationFunctionType.Relu)
                nc.sync.dma_start(
                    out=bf[:, ch * (RCH // 2):(ch + 1) * (RCH // 2), :],
                    in_=outc[:])

            # RV band, 2 rows packed in M
            trv = rvp.tile([128, rv_halo // 2, rv_w], BF16, tag="trv")
            nc.sync.dma_start(out=trv[:], in_=grv[:])
            CW = 512
            for pr in range(nrv_rows // 2):
                r = 2 * pr
                outr = ob.tile([2 * cout, rv_w], F32, tag="outr")
                for cwi in range(rv_w // CW):
                    acc = ps.tile([2 * cout, CW], F32, tag="racc")
                    base = cwi * CW
                    nmm = 0
                    for pa in range(2):
                        for dx in range(3):
                            lo = base + dx - 1
                            dst0 = max(0, -lo)
                            src0 = max(0, lo)
                            w = min(CW - dst0, rv_w - src0)
                            nc.tensor.matmul(
                                out=acc[:, dst0:dst0 + w], lhsT=krvs[:, pa, dx, :],
                                rhs=trv[:, pr + pa, src0:src0 + w],
                                start=(nmm == 0), stop=(nmm == 5))
                            nmm += 1
                    nc.vector.tensor_copy(out=outr[:, base:base + CW], in_=acc[:])
                relu_r = ob.tile([2 * cout, rv_w], F32, tag="relur")
                nc.scalar.activation(relu_r[:], outr[:],
                                     mybir.ActivationFunctionType.Relu)
                nc.sync.dma_start(out=rf[:, pr, :], in_=relu_r[:])
    return nc


# ---------------------------------------------------------------- launch C
def _build_fusion(np_pts):
    """pred[3, np] = w_pred @ relu(w_fuse @ fused[192, np])"""
    nc = bacc.Bacc("TRN2", target_bir_lowering=False)
    f_lo = nc.dram_tensor("f_lo", [128, np_pts], BF16, kind="ExternalInput")
    f_hi = nc.dram_tensor("f_hi", [64, np_pts], BF16, kind="ExternalInput")
    wft_lo = nc.dram_tensor("wft_lo", [128, FP], BF16, kind="ExternalInput")
    wft_hi = nc.dram_tensor("wft_hi", [64, FP], BF16, kind="ExternalInput")
    wpt = nc.dram_tensor("wpt", [FP, 4], BF16, kind="ExternalInput")
    pred = nc.dram_tensor("pred", [4, np_pts], F32, kind="ExternalOutput")
    CH = 8192
    with tile.TileContext(nc) as tc:
        with ExitStack() as ctx:
            sb = ctx.enter_context(tc.tile_pool(name="sb", bufs=2))
            wp = ctx.enter_context(tc.tile_pool(name="wp", bufs=1))
            ps = ctx.enter_context(tc.tile_pool(name="ps", bufs=4, space="PSUM"))
            wlo = wp.tile([128, FP], BF16)
            nc.sync.dma_start(out=wlo[:], in_=wft_lo[:])
            whi = wp.tile([64, FP], BF16)
            nc.sync.dma_start(out=whi[:], in_=wft_hi[:])
            wps = wp.tile([FP, 4], BF16)
            nc.sync.dma_start(out=wps[:], in_=wpt[:])
            for ci in range(np_pts // CH):
                xlo = sb.tile([128, CH], BF16, tag="xlo")
                nc.sync.dma_start(out=xlo[:], in_=f_lo[:, bass.ts(ci, CH)])
                xhi = sb.tile([64, CH], BF16, tag="xhi")
                nc.sync.dma_start(out=xhi[:], in_=f_hi[:, bass.ts(ci, CH)])
                pout = sb.tile([4, CH], F32, tag="pout")
                for t in range(CH // 512):
                    p1 = ps.tile([FP, 512], F32, tag="p1")
                    nc.tensor.matmul(out=p1[:], lhsT=wlo[:],
                                     rhs=xlo[:, bass.ts(t, 512)],
                                     start=True, stop=False)
                    nc.tensor.matmul(out=p1[:], lhsT=whi[:],
                                     rhs=xhi[:, bass.ts(t, 512)],
                                     start=False, stop=True)
                    pf_t = sb.tile([FP, 512], BF16, tag="pf_t")
                    nc.scalar.activation(pf_t[:], p1[:],
                                         mybir.ActivationFunctionType.Relu)
                    p2 = ps.tile([4, 512], F32, tag="p2")
                    nc.tensor.matmul(out=p2[:], lhsT=wps[:],
                                     rhs=pf_t[:],
                                     start=True, stop=True)
                    nc.vector.tensor_copy(out=pout[:, bass.ts(t, 512)], in_=p2[:])
                nc.sync.dma_start(out=pred[:, bass.ts(ci, CH)], in_=pout[:])
    return nc


# ---------------------------------------------------------------- host glue
def _rowpair(g):
    """[64, R, W] -> [128, R//2, W]: partition 64*b+ch = row parity b."""
    c, r, w = g.shape
    out = np.empty((2 * c, r // 2, w), g.dtype)
    out[:c] = g[:, 0::2, :]
    out[c:] = g[:, 1::2, :]
    return np.ascontiguousarray(out)


def _segment_max(vals_pm, idx, ncells):
    """vals_pm [n, 64] f32, idx [n] -> grid [ncells, 64] (empty cells 0)."""
    order = np.argsort(idx, kind="stable")
    si = idx[order]
    sv = vals_pm[order]
    starts = np.concatenate(([0], np.flatnonzero(np.diff(si)) + 1))
    seg = np.maximum.reduceat(sv, starts, axis=0)
    grid = np.zeros((ncells, vals_pm.shape[1]), np.float32)
    grid[si[starts]] = seg
    return grid


def kernel(**inputs):
    pf_full = np.ascontiguousarray(
        inputs["point_feat"][..., 0].reshape(BS * T, C, N).astype(np.float32))
    coord = inputs["pcds_coord"][..., 0].reshape(BS * T, N, 3)
    sph = inputs["pcds_sphere_coord"][:, 0, :, :, 0]
    w1, w2 = inputs["w_pre1"], inputs["w_pre2"]
    k_bev, k_rv = inputs["k_bev"], inputs["k_rv"]
    w_fuse, w_pred = inputs["w_fuse"], inputs["w_pred"]

    bev_idx = (coord[:, :, 0].astype(np.int64) * BEV_W + coord[:, :, 1]).astype(np.int64)
    rv_idx = (sph[:, :, 0].astype(np.int64) * RV_W + sph[:, :, 1]).astype(np.int64)

    # ---- launch A: MLP over all 6*N points, sharded 8 ways
    NTOT = BS * T * N                      # 780000
    NPAD = ((NTOT + NCORES * 8192 - 1) // (NCORES * 8192)) * (NCORES * 8192)
    per = NPAD // NCORES                   # points per core (mult of 8192)
    pf_flat = np.zeros((C, NPAD), np.float32)
    pf_flat[:, :NTOT] = pf_full.transpose(1, 0, 2).reshape(C, NTOT)
    nc_a = _build_mlp(per, None)
    w1t = np.ascontiguousarray(w1.T).astype(ml_dtypes.bfloat16)
    w2t = np.ascontiguousarray(w2.T).astype(ml_dtypes.bfloat16)
    pf_bf = pf_flat.astype(ml_dtypes.bfloat16)
    maps = [{"pf": np.ascontiguousarray(pf_bf[:, i * per:(i + 1) * per]),
             "w1t": w1t, "w2t": w2t} for i in range(NCORES)]
    res = _run(nc_a, maps)
    h_flat = np.concatenate([np.asarray(res[i]["h"]) for i in range(NCORES)],
                            axis=1)[:, :NTOT]
    h = h_flat.reshape(FP, BS * T, N).transpose(1, 0, 2)   # [6, 64, N] f32

    # ---- host: scatter-max (sorted segment max), residual stack
    h_bf = h.astype(ml_dtypes.bfloat16).astype(np.float32)
    grids = np.stack([
        _segment_max(h_bf[f].T, bev_idx[f], BEV_H * BEV_W).T
        for f in range(BS * T)]).reshape(BS, T, FP, BEV_H, BEV_W)
    base = grids[:, 0]
    cat = np.concatenate([base[:, None], grids[:, 1:] - base[:, None]],
                         axis=1).reshape(BS, T * FP, BEV_H, BEV_W)
    h_cur = h.reshape(BS, T, FP, N)[:, 0]                   # f32
    rv_grids = np.stack([
        _segment_max(h_cur[s].astype(ml_dtypes.bfloat16).astype(np.float32).T,
                     rv_idx[s], RV_H * RV_W).T
        for s in range(BS)]).reshape(BS, FP, RV_H, RV_W)

    # ---- launch B: convs, core = (sample, band of 128 BEV rows + 16 RV rows)
    BAND, RVB = BEV_H // 4, RV_H // 4
    katap9 = k_bev.transpose(1, 2, 3, 0).reshape(192, 3, 3, FP).astype(np.float32)
    kpair = np.zeros((192, 4, 3, 2 * FP), np.float32)
    for j in range(4):
        if j <= 2:
            kpair[:, j, :, :FP] = katap9[:, j, :, :]       # row r, dy=j
        if j >= 1:
            kpair[:, j, :, FP:] = katap9[:, j - 1, :, :]   # row r+1, dy=j-1
    kpair = np.ascontiguousarray(kpair).astype(ml_dtypes.bfloat16)
    # hi group: K packs 2 input rows (j = 2*pa + b); M packs 2 output rows
    khi2 = np.zeros((128, 2, 3, 2 * FP), np.float32)
    kat_hi = katap9[128:]                              # [64, 3, 3, FP]
    for pa in range(2):
        for b in range(2):
            j = 2 * pa + b
            if j <= 2:
                khi2[64 * b:64 * (b + 1), pa, :, :FP] = kat_hi[:, j, :, :]
            if 1 <= j <= 3:
                khi2[64 * b:64 * (b + 1), pa, :, FP:] = kat_hi[:, j - 1, :, :]
    khi2 = np.ascontiguousarray(khi2).astype(ml_dtypes.bfloat16)
    krtap9 = k_rv.transpose(1, 2, 3, 0).reshape(64, 3, 3, FP).astype(np.float32)
    krpair = np.zeros((64, 4, 3, 2 * FP), np.float32)
    for j in range(4):
        if j <= 2:
            krpair[:, j, :, :FP] = krtap9[:, j, :, :]
        if j >= 1:
            krpair[:, j, :, FP:] = krtap9[:, j - 1, :, :]
    krv2 = np.zeros((128, 2, 3, 2 * FP), np.float32)
    for pa in range(2):
        for b in range(2):
            j = 2 * pa + b
            if j <= 2:
                krv2[64 * b:64 * (b + 1), pa, :, :FP] = krtap9[:, j, :, :]
            if 1 <= j <= 3:
                krv2[64 * b:64 * (b + 1), pa, :, FP:] = krtap9[:, j - 1, :, :]
    krv2 = np.ascontiguousarray(krv2).astype(ml_dtypes.bfloat16)
    cat_pad = np.pad(cat, ((0, 0), (0, 0), (1, 1), (0, 0))).astype(ml_dtypes.bfloat16)
    rv_pad = np.pad(rv_grids, ((0, 0), (0, 0), (1, 1), (0, 0))).astype(ml_dtypes.bfloat16)
    maps = []
    for core in range(NCORES):
        s, q = divmod(core, 4)
        r0 = q * BAND
        maps.append({
            "glo": np.ascontiguousarray(cat_pad[s, :128, r0:r0 + BAND + 2, :]),
            "ghi": _rowpair(cat_pad[s, 128:, r0:r0 + BAND + 2, :]),
            "klo": kpair[:128], "khi": khi2,
            "grv": _rowpair(rv_pad[s, :, q * RVB:q * RVB + RVB + 2, :]),
            "krv": krv2,
        })
    nc_b = _build_conv(BAND, BAND + 2, BEV_W, 128, 64, FP, RVB, RVB + 2, RV_W)
    res = _run(nc_b, maps)
    bev_feat = np.zeros((BS, FP, BEV_H, BEV_W), np.float32)
    rv_feat = np.zeros((BS, FP, RV_H, RV_W), np.float32)
    for core in range(NCORES):
        s, q = divmod(core, 4)
        bfp = np.asarray(res[core]["bf"])            # [128, BAND//2, W]
        bev_feat[s, :, q * BAND + 0:(q + 1) * BAND:2, :] = bfp[:FP]
        bev_feat[s, :, q * BAND + 1:(q + 1) * BAND:2, :] = bfp[FP:]
        rfp = np.asarray(res[core]["rf"])            # [128, RVB//2, RVW]
        rv_feat[s, :, q * RVB + 0:(q + 1) * RVB:2, :] = rfp[:FP]
        rv_feat[s, :, q * RVB + 1:(q + 1) * RVB:2, :] = rfp[FP:]

    # ---- host gathers
    cur_idx = bev_idx.reshape(BS, T, N)[:, 0]
    point_bev = np.take_along_axis(bev_feat.reshape(BS, FP, -1),
                                   cur_idx[:, None, :], axis=2)
    point_rv = np.take_along_axis(rv_feat.reshape(BS, FP, -1),
                                  rv_idx[:, None, :], axis=2)
    fused = np.concatenate([h_cur, point_bev, point_rv], axis=1)  # [BS,192,N]

    # ---- launch C: fusion
    NT2 = BS * N
    NP2 = ((NT2 + NCORES * 8192 - 1) // (NCORES * 8192)) * (NCORES * 8192)
    per2 = NP2 // NCORES
    fused_flat = np.zeros((192, NP2), np.float32)
    fused_flat[:, :NT2] = fused.transpose(1, 0, 2).reshape(192, NT2)
    fused_flat = fused_flat.astype(ml_dtypes.bfloat16)
    wft = w_fuse.T.astype(ml_dtypes.bfloat16)          # [192, 64]
    wpt = np.zeros((FP, 4), np.float32)
    wpt[:, :3] = w_pred.T
    wpt = wpt.astype(ml_dtypes.bfloat16)
    nc_c = _build_fusion(per2)
    maps = [{"f_lo": np.ascontiguousarray(fused_flat[:128, i * per2:(i + 1) * per2]),
             "f_hi": np.ascontiguousarray(fused_flat[128:, i * per2:(i + 1) * per2]),
             "wft_lo": np.ascontiguousarray(wft[:128]),
             "wft_hi": np.ascontiguousarray(wft[128:]),
             "wpt": wpt} for i in range(NCORES)]
    res = _run(nc_c, maps)
    pred_flat = np.concatenate([np.asarray(res[i]["pred"]) for i in range(NCORES)],
                               axis=1)[:3, :NT2]
    pred = pred_flat.reshape(3, BS, N).transpose(1, 0, 2)[..., None]
    return pred.astype(np.float32)

